# revision 29
# baseline (speedup 1.0000x reference)
"""GA3 Conv2d kernel for 8 Trainium2 NeuronCores — Winograd F(2,3) along H.

Math: the sign-combination einsum folds into the conv weights, making the
module ONE dense 3x3 conv with Cin=Cout=128 on [B, 128, 128, 128] (see
_prep math below).  We shard data-parallel over B (1 image per core).

The direct 9-tap matmul formulation is PE-bound: 9 taps x 512 cols x 32
groups = 147k PE cycles, ~70 us at the power-throttled ~2.1 GHz clock.
Winograd F(2,3) applied along H cuts PE work to 2/3: for each pair of
output rows (2c, 2c+1) and input rows d0..d3 = padded rows 2c..2c+3,
    m0 = conv_w(t0) @ G0,  t0 = d0 - d2,   G0 = W[dh=0]
    m1 = conv_w(t1) @ G1,  t1 = d1 + d2,   G1 = (W0+W1+W2)/2
    m2 = conv_w(t2) @ G2,  t2 = d2 - d1,   G2 = (W0-W1+W2)/2
    m3 = conv_w(t3) @ G3,  t3 = d1 - d3,   G3 = W[dh=2]
    even row = m0 + m1 + m2 + bias ;  odd row = m1 - m2 - m3 + bias
where conv_w is the remaining direct 3-tap conv along W (dw = 0..2).
The row transforms t0..t3 are LINEAR and cheap -> computed on HOST and
shipped as fp16 planes (2x input bytes, still under the PE roofline).
The output combine runs on DVE (even rows) and GPSIMD (odd rows) as two
fused scalar_tensor_tensor passes each, overlapping the PE stream; the
bias rides the STT per-partition scalar operand for free.

Layout: per tile c (c = 0..63) the host packs 4 width-padded rows
[t0 | t3 | t1 | t2], each [1 pad][128 data][1 pad] = 130 elems, so every
matmul rhs is a [520-stride x 4 tiles][1 x 128] AP and all loads are
contiguous chunks.  Even/odd output rows go to separate DRAM planes in
fp16; the host interleaves and upcasts (measured rel err ~5e-4).
"""

import numpy as np

_TERMS = [
    [(0, 0, 1), (1, 1, 1), (2, 2, 1), (3, 3, 1), (4, 4, -1), (5, 5, -1), (6, 6, -1), (7, 7, -1)],
    [(1, 0, 1), (0, 1, 1), (2, 4, 1), (4, 2, -1), (3, 6, 1), (6, 3, -1), (5, 7, -1), (7, 5, -1)],
    [(2, 0, 1), (0, 2, 1), (1, 4, -1), (4, 1, 1), (3, 5, 1), (5, 3, -1), (6, 7, 1), (7, 6, 1)],
    [(3, 0, 1), (0, 3, 1), (1, 6, -1), (6, 1, 1), (2, 5, -1), (5, 2, 1), (4, 7, -1), (7, 4, -1)],
    [(4, 0, 1), (0, 4, 1), (2, 1, 1), (1, 2, -1), (3, 7, 1), (7, 3, 1), (6, 5, 1), (5, 6, -1)],
    [(5, 0, 1), (0, 5, 1), (3, 2, 1), (2, 3, -1), (1, 7, 1), (7, 1, 1), (4, 6, 1), (6, 4, -1)],
    [(6, 0, 1), (0, 6, 1), (3, 1, 1), (1, 3, -1), (2, 7, -1), (7, 2, -1), (5, 4, 1), (4, 5, -1)],
    [(7, 0, 1), (0, 7, 1), (5, 1, 1), (1, 5, 1), (6, 2, -1), (2, 6, -1), (4, 3, 1), (3, 4, 1)],
]
_S = np.zeros((8, 8, 8), dtype=np.float32)
for _m, _terms in enumerate(_TERMS):
    for _j, _k, _s in _terms:
        _S[_m, _j, _k] = _s

B, CIN, COUT, H, W = 8, 16, 16, 128, 128
C = 8 * CIN          # 128 interleaved channels
N_CORES = 8
NT = H // 2          # 64 row-tiles (one per output row pair)
PWR = W + 2          # padded row: [pad][128][pad]
TBLK = 4 * PWR       # per-tile block: rows [t0, t3, t1, t2]
FLAT = NT * TBLK     # flat elems/partition
ROWOFF = [0, 2 * PWR, 3 * PWR, PWR]   # plane offset of t_u within a block
GPT = 4              # tiles per PSUM group (4 tiles = 4 even + 4 odd rows)
NG = NT // GPT       # 16 groups
FD = GPT * W         # 512 matmul free dim / combine span
WCOLS = 12 * C + 1   # 12 transformed weight mats + bias column
N_WARMUP = 11        # HAM warm-up matmuls during the head DMAs

_CACHED_NC = None


def _build_nc():
    import concourse.bass as bass
    import concourse.mybir as mybir
    import concourse.tile as tile
    from concourse import bacc

    f32 = mybir.dt.float32
    f16 = mybir.dt.float16
    ADD = mybir.AluOpType.add
    SUB = mybir.AluOpType.subtract
    MUL = mybir.AluOpType.mult

    nc = bacc.Bacc("TRN2", target_bir_lowering=False, debug=False,
                   enable_asserts=False)

    xb = nc.dram_tensor("xb", [C, FLAT], f16, kind="ExternalInput").ap()
    wf = nc.dram_tensor("wf", [C, WCOLS], f16, kind="ExternalInput").ap()
    bb = nc.dram_tensor("bb", [C, 1], f32, kind="ExternalInput").ap()
    oute = nc.dram_tensor("oute", [C, NT, W], f16, kind="ExternalOutput").ap()
    outo = nc.dram_tensor("outo", [C, NT, W], f16, kind="ExternalOutput").ap()

    with tile.TileContext(nc) as tc:
        with (
            tc.tile_pool(name="wpool", bufs=1) as wpool,
            tc.tile_pool(name="xpool", bufs=1) as xpool,
            tc.tile_pool(name="pspool", bufs=2, space="PSUM") as pspool,
            tc.tile_pool(name="tpool", bufs=8) as tpool,
            tc.tile_pool(name="opool", bufs=6) as opool,
        ):
            xfull = xpool.tile([C, FLAT], f16)
            wtile = wpool.tile([C, WCOLS], f16)

            # All loads ride the SP ring in FIFO order, weights first (the
            # DMA queues share the physical engine, so a "parallel" ring
            # only queues behind).  Stores get the ACT ring.  No PE warm-up
            # matmuls: traces show they serialize on ~0.5us semaphore waits
            # and only delay the first real matmul.
            nc.sync.dma_start(out=wtile[:, :], in_=wf[:, :])
            btile = wpool.tile([C, 1], f32)
            nc.sync.dma_start(out=btile[:, :], in_=bb[:, :])

            # input chunks: one group's 4 tile-blocks each (contiguous)
            def emit_chunk(g):
                lo, hi = g * GPT * TBLK, (g + 1) * GPT * TBLK
                nc.sync.dma_start(out=xfull[:, lo:hi], in_=xb[:, lo:hi])

            for g in range(4):
                emit_chunk(g)

            # Engine constraints: GPSIMD cannot touch PSUM; DVE may read only
            # ONE PSUM operand per op; ACT reads PSUM fine (its activation
            # copy measured ~0.7us).  Combine dataflow per group:
            #   DVE: s1 = m1 + bias ; tmp_e = m0 + s1 ; obuf_o = tmp_o - m3
            #   ACT: s2 = m2
            #   GP : tmp_o = s1 - s2 ; obuf_e = tmp_e + s2
            # Every engine drains its queue strictly in order, so an op that
            # waits on ANOTHER engine blocks everything emitted after it on
            # the same queue.  To kill that head-of-line blocking the
            # combine is SOFTWARE-PIPELINED: stage A (s1/s2/tmp_e/tmp_o,
            # intra-group deps only) is emitted with group g's matmuls,
            # stage B (obuf_e/obuf_o/stores, cross-engine deps) is emitted
            # one group later, when its inputs are already done.

            def stage_b(t, last=False):
                g = t["g"]
                obuf_e = opool.tile([C, FD], f16, name="obuf_e")
                obuf_o = opool.tile([C, FD], f16, name="obuf_o")
                nc.vector.scalar_tensor_tensor(
                    out=obuf_o[:, :], in0=t["ps3"][:, :], scalar=-1.0,
                    in1=t["tmp_o"][:, :], op0=MUL, op1=ADD)
                nc.gpsimd.tensor_add(obuf_e[:, :], t["tmp_e"][:, :],
                                     t["s2"][:, :])
                if not last:
                    # even-row stores on the ACT ring, odd-row on SP
                    nc.scalar.dma_start(out=oute[:, g * GPT:(g + 1) * GPT, :],
                                        in_=obuf_e[:, :])
                    nc.sync.dma_start(out=outo[:, g * GPT:(g + 1) * GPT, :],
                                      in_=obuf_o[:, :])
                else:
                    # split the final stores so the tail barrier waits on
                    # small last transfers
                    for t_ in range(GPT):
                        nc.scalar.dma_start(
                            out=oute[:, g * GPT + t_, :],
                            in_=obuf_e[:, t_ * W:(t_ + 1) * W])
                        nc.sync.dma_start(
                            out=outo[:, g * GPT + t_, :],
                            in_=obuf_o[:, t_ * W:(t_ + 1) * W])

            pending = None
            for g in range(NG):
                if g < NG - 4:
                    emit_chunk(g + 4)
                ps = [pspool.tile([C, FD], f32, name=f"psu{u}", tag=f"ps{u}")
                      for u in range(4)]
                for u in (1, 2, 0, 3):
                    for dw in range(3):
                        base = g * GPT * TBLK + ROWOFF[u] + dw
                        rhs = bass.AP(xfull.tensor, xfull.offset + base,
                                      [xfull.ap[0], [TBLK, GPT], [1, W]])
                        nc.tensor.matmul(
                            ps[u][:, :],
                            lhsT=wtile[:, (u * 3 + dw) * C:(u * 3 + dw + 1) * C],
                            rhs=rhs,
                            start=(dw == 0),
                            stop=(dw == 2),
                        )
                s1 = tpool.tile([C, FD], f16)
                s2 = tpool.tile([C, FD], f16)
                tmp_e = tpool.tile([C, FD], f16)
                tmp_o = tpool.tile([C, FD], f16)
                nc.vector.tensor_scalar_add(out=s1[:, :], in0=ps[1][:, :],
                                            scalar1=btile[:, 0:1])
                nc.scalar.copy(out=s2[:, :], in_=ps[2][:, :])
                nc.vector.scalar_tensor_tensor(
                    out=tmp_e[:, :], in0=ps[0][:, :], scalar=0.0,
                    in1=s1[:, :], op0=ADD, op1=ADD)
                nc.gpsimd.tensor_sub(tmp_o[:, :], s1[:, :], s2[:, :])
                if pending is not None:
                    stage_b(pending)
                pending = {"g": g, "ps3": ps[3], "s2": s2,
                           "tmp_e": tmp_e, "tmp_o": tmp_o}
            stage_b(pending, last=True)

    nc.compile()
    return nc


def _get_nc():
    global _CACHED_NC
    if _CACHED_NC is None:
        _CACHED_NC = _build_nc()
    return _CACHED_NC


def _prep_weights(Wfull: np.ndarray, b: np.ndarray):
    # V[ci*8+k, dh, dw, co*8+m] = sum_j S[m,j,k] * W[j, co, ci, dh, dw]
    V = np.einsum("mjk,jcihw->ikhwcm", _S.astype(np.float64),
                  np.asarray(Wfull).astype(np.float64)).reshape(C, 3, 3, C)
    G = [V[:, 0], (V[:, 0] + V[:, 1] + V[:, 2]) / 2,
         (V[:, 0] - V[:, 1] + V[:, 2]) / 2, V[:, 2]]   # each [ic, dw, oc]
    wf = np.empty((C, WCOLS), dtype=np.float16)
    for u in range(4):
        for dw in range(3):
            wf[:, (u * 3 + dw) * C:(u * 3 + dw + 1) * C] = G[u][:, dw, :]
    bias = np.einsum("mjk,jc->cm", _S.astype(np.float64),
                     np.asarray(b).astype(np.float64)).reshape(C)
    wf[:, 12 * C] = bias.astype(np.float16)
    return np.ascontiguousarray(wf), bias.astype(np.float32).reshape(C, 1)


def _prep_inputs(x: np.ndarray) -> np.ndarray:
    # [B, C, H, W] -> Winograd row-transformed flat planes [B, C, FLAT]
    nB = x.shape[0]
    pr = np.zeros((nB, C, H + 2, W), dtype=np.float32)
    pr[:, :, 1:-1, :] = x
    xt = np.zeros((nB, C, NT, 4, PWR), dtype=np.float16)
    xt[:, :, :, 0, 1:W + 1] = pr[:, :, 0:2 * NT:2] - pr[:, :, 2:2 * NT + 2:2]
    xt[:, :, :, 1, 1:W + 1] = pr[:, :, 1:2 * NT + 1:2] - pr[:, :, 3:2 * NT + 3:2]
    xt[:, :, :, 2, 1:W + 1] = pr[:, :, 1:2 * NT + 1:2] + pr[:, :, 2:2 * NT + 2:2]
    xt[:, :, :, 3, 1:W + 1] = pr[:, :, 2:2 * NT + 2:2] - pr[:, :, 1:2 * NT + 1:2]
    return xt.reshape(nB, C, FLAT)


def kernel(x: np.ndarray, W: np.ndarray, b: np.ndarray) -> np.ndarray:
    from concourse.bass_utils import run_bass_kernel_spmd

    xt = _prep_inputs(np.ascontiguousarray(x, dtype=np.float32))
    wf, bb = _prep_weights(W, b)

    nc = _get_nc()
    in_maps = [{"xb": xt[c], "wf": wf, "bb": bb} for c in range(N_CORES)]
    res = run_bass_kernel_spmd(nc, in_maps, core_ids=list(range(N_CORES)))
    out = np.empty((N_CORES, C, H, 128), dtype=np.float32)
    for c in range(N_CORES):
        out[c, :, 0::2, :] = res.results[c]["oute"].astype(np.float32)
        out[c, :, 1::2, :] = res.results[c]["outo"].astype(np.float32)
    return out


# revision 33
# speedup vs baseline: 1.0671x; 1.0671x over previous
"""GA3 Conv2d kernel for 8 Trainium2 NeuronCores — Winograd F(2,3) along H.

Math: the sign-combination einsum folds into the conv weights, making the
module ONE dense 3x3 conv with Cin=Cout=128 on [B, 128, 128, 128] (see
_prep math below).  We shard data-parallel over B (1 image per core).

The direct 9-tap matmul formulation is PE-bound: 9 taps x 512 cols x 32
groups = 147k PE cycles, ~70 us at the power-throttled ~2.1 GHz clock.
Winograd F(2,3) applied along H cuts PE work to 2/3: for each pair of
output rows (2c, 2c+1) and input rows d0..d3 = padded rows 2c..2c+3,
    m0 = conv_w(t0) @ G0,  t0 = d0 - d2,   G0 = W[dh=0]
    m1 = conv_w(t1) @ G1,  t1 = d1 + d2,   G1 = (W0+W1+W2)/2
    m2 = conv_w(t2) @ G2,  t2 = d2 - d1,   G2 = (W0-W1+W2)/2
    m3 = conv_w(t3) @ G3,  t3 = d1 - d3,   G3 = W[dh=2]
    even row = m0 + m1 + m2 + bias ;  odd row = m1 - m2 - m3 + bias
where conv_w is the remaining direct 3-tap conv along W (dw = 0..2).
The row transforms t0..t3 are LINEAR and cheap -> computed on HOST and
shipped as fp16 planes (2x input bytes, still under the PE roofline).
The output combine runs on DVE (even rows) and GPSIMD (odd rows) as two
fused scalar_tensor_tensor passes each, overlapping the PE stream; the
bias rides the STT per-partition scalar operand for free.

Layout: per tile c (c = 0..63) the host packs 4 width-padded rows
[t0 | t3 | t1 | t2], each [1 pad][128 data][1 pad] = 130 elems, so every
matmul rhs is a [520-stride x 4 tiles][1 x 128] AP and all loads are
contiguous chunks.  Even/odd output rows go to separate DRAM planes in
fp16; the host interleaves and upcasts (measured rel err ~5e-4).
"""

import numpy as np

_TERMS = [
    [(0, 0, 1), (1, 1, 1), (2, 2, 1), (3, 3, 1), (4, 4, -1), (5, 5, -1), (6, 6, -1), (7, 7, -1)],
    [(1, 0, 1), (0, 1, 1), (2, 4, 1), (4, 2, -1), (3, 6, 1), (6, 3, -1), (5, 7, -1), (7, 5, -1)],
    [(2, 0, 1), (0, 2, 1), (1, 4, -1), (4, 1, 1), (3, 5, 1), (5, 3, -1), (6, 7, 1), (7, 6, 1)],
    [(3, 0, 1), (0, 3, 1), (1, 6, -1), (6, 1, 1), (2, 5, -1), (5, 2, 1), (4, 7, -1), (7, 4, -1)],
    [(4, 0, 1), (0, 4, 1), (2, 1, 1), (1, 2, -1), (3, 7, 1), (7, 3, 1), (6, 5, 1), (5, 6, -1)],
    [(5, 0, 1), (0, 5, 1), (3, 2, 1), (2, 3, -1), (1, 7, 1), (7, 1, 1), (4, 6, 1), (6, 4, -1)],
    [(6, 0, 1), (0, 6, 1), (3, 1, 1), (1, 3, -1), (2, 7, -1), (7, 2, -1), (5, 4, 1), (4, 5, -1)],
    [(7, 0, 1), (0, 7, 1), (5, 1, 1), (1, 5, 1), (6, 2, -1), (2, 6, -1), (4, 3, 1), (3, 4, 1)],
]
_S = np.zeros((8, 8, 8), dtype=np.float32)
for _m, _terms in enumerate(_TERMS):
    for _j, _k, _s in _terms:
        _S[_m, _j, _k] = _s

B, CIN, COUT, H, W = 8, 16, 16, 128, 128
C = 8 * CIN          # 128 interleaved channels
N_CORES = 8
NT = H // 2          # 64 row-tiles (one per output row pair)
PWR = W + 2          # padded row: [pad][128][pad]
TBLK = 4 * PWR       # per-tile block: rows [t0, t3, t1, t2]
FLAT = NT * TBLK     # flat elems/partition
ROWOFF = [0, 2 * PWR, 3 * PWR, PWR]   # plane offset of t_u within a block
GPT = 4              # tiles per PSUM group (4 tiles = 4 even + 4 odd rows)
NG = NT // GPT       # 16 groups
FD = GPT * W         # 512 matmul free dim / combine span
WCOLS = 12 * C + 1   # 12 transformed weight mats + bias column
N_WARMUP = 11        # HAM warm-up matmuls during the head DMAs

_CACHED_NC = None


def _build_nc():
    import concourse.bass as bass
    import concourse.mybir as mybir
    import concourse.tile as tile
    from concourse import bacc

    f32 = mybir.dt.float32
    f16 = mybir.dt.float16
    ADD = mybir.AluOpType.add
    SUB = mybir.AluOpType.subtract
    MUL = mybir.AluOpType.mult

    nc = bacc.Bacc("TRN2", target_bir_lowering=False, debug=False,
                   enable_asserts=False)

    xb = nc.dram_tensor("xb", [C, FLAT], f16, kind="ExternalInput").ap()
    wf = nc.dram_tensor("wf", [C, WCOLS], f16, kind="ExternalInput").ap()
    bb = nc.dram_tensor("bb", [C, 1], f32, kind="ExternalInput").ap()
    # unified output: [C, group, parity, tile-in-group, W] so each group's
    # even+odd rows leave in ONE contiguous [C, 1024] store
    out2 = nc.dram_tensor("out2", [C, NG, 2, GPT, W], f16,
                          kind="ExternalOutput").ap()

    with tile.TileContext(nc) as tc:
        with (
            tc.tile_pool(name="wpool", bufs=1) as wpool,
            tc.tile_pool(name="xpool", bufs=1) as xpool,
            tc.tile_pool(name="pspool", bufs=2, space="PSUM") as pspool,
            tc.tile_pool(name="tpool", bufs=8) as tpool,
            tc.tile_pool(name="opool", bufs=4) as opool,
        ):
            xfull = xpool.tile([C, FLAT], f16)
            wtile = wpool.tile([C, WCOLS], f16)

            # All loads ride the SP ring in FIFO order, weights first (the
            # DMA queues share the physical engine, so a "parallel" ring
            # only queues behind).  Stores get the ACT ring.  No PE warm-up
            # matmuls: traces show they serialize on ~0.5us semaphore waits
            # and only delay the first real matmul.
            nc.sync.dma_start(out=wtile[:, :], in_=wf[:, :])
            btile = wpool.tile([C, 1], f32)
            nc.sync.dma_start(out=btile[:, :], in_=bb[:, :])

            # input chunks: one group's 4 tile-blocks each (contiguous)
            def emit_chunk(g):
                lo, hi = g * GPT * TBLK, (g + 1) * GPT * TBLK
                nc.sync.dma_start(out=xfull[:, lo:hi], in_=xb[:, lo:hi])

            for g in range(4):
                emit_chunk(g)

            # Engine constraints: GPSIMD cannot touch PSUM; DVE may read only
            # ONE PSUM operand per op; ACT reads PSUM fine (its activation
            # copy measured ~0.7us).  Combine dataflow per group:
            #   DVE: s1 = m1 + bias ; tmp_e = m0 + s1 ; obuf_o = tmp_o - m3
            #   ACT: s2 = m2
            #   GP : tmp_o = s1 - s2 ; obuf_e = tmp_e + s2
            # Every engine drains its queue strictly in order, so an op that
            # waits on ANOTHER engine blocks everything emitted after it on
            # the same queue.  To kill that head-of-line blocking the
            # combine is SOFTWARE-PIPELINED: stage A (s1/s2/tmp_e/tmp_o,
            # intra-group deps only) is emitted with group g's matmuls,
            # stage B (obuf_e/obuf_o/stores, cross-engine deps) is emitted
            # one group later, when its inputs are already done.

            def stage_b(t, last=False):
                g = t["g"]
                obuf = opool.tile([C, 2 * FD], f16, name="obuf")
                nc.gpsimd.tensor_add(obuf[:, 0:FD], t["tmp_e"][:, :],
                                     t["s2"][:, :])
                nc.vector.scalar_tensor_tensor(
                    out=obuf[:, FD:2 * FD], in0=t["ps3"][:, :], scalar=-1.0,
                    in1=t["tmp_o"][:, :], op0=MUL, op1=ADD)
                if not last:
                    # one store per group on the otherwise-idle ACT ring
                    nc.scalar.dma_start(out=out2[:, g, :, :, :],
                                        in_=obuf[:, :])
                else:
                    # split the final stores so the tail barrier waits on
                    # small last transfers
                    nc.scalar.dma_start(out=out2[:, g, 0, :, :],
                                        in_=obuf[:, 0:FD])
                    nc.scalar.dma_start(out=out2[:, g, 1, :, :],
                                        in_=obuf[:, FD:2 * FD])

            pending = None
            for g in range(NG):
                if g < NG - 4:
                    emit_chunk(g + 4)
                ps = [pspool.tile([C, FD], f32, name=f"psu{u}", tag=f"ps{u}")
                      for u in range(4)]
                for u in (1, 2, 0, 3):
                    for dw in range(3):
                        base = g * GPT * TBLK + ROWOFF[u] + dw
                        rhs = bass.AP(xfull.tensor, xfull.offset + base,
                                      [xfull.ap[0], [TBLK, GPT], [1, W]])
                        nc.tensor.matmul(
                            ps[u][:, :],
                            lhsT=wtile[:, (u * 3 + dw) * C:(u * 3 + dw + 1) * C],
                            rhs=rhs,
                            start=(dw == 0),
                            stop=(dw == 2),
                        )
                s1 = tpool.tile([C, FD], f16)
                s2 = tpool.tile([C, FD], f16)
                tmp_e = tpool.tile([C, FD], f16)
                tmp_o = tpool.tile([C, FD], f16)
                nc.vector.tensor_scalar_add(out=s1[:, :], in0=ps[1][:, :],
                                            scalar1=btile[:, 0:1])
                nc.scalar.copy(out=s2[:, :], in_=ps[2][:, :])
                nc.vector.scalar_tensor_tensor(
                    out=tmp_e[:, :], in0=ps[0][:, :], scalar=0.0,
                    in1=s1[:, :], op0=ADD, op1=ADD)
                nc.gpsimd.tensor_sub(tmp_o[:, :], s1[:, :], s2[:, :])
                if pending is not None:
                    stage_b(pending)
                pending = {"g": g, "ps3": ps[3], "s2": s2,
                           "tmp_e": tmp_e, "tmp_o": tmp_o}
            stage_b(pending, last=True)

    nc.compile()
    return nc


def _get_nc():
    global _CACHED_NC
    if _CACHED_NC is None:
        _CACHED_NC = _build_nc()
    return _CACHED_NC


def _prep_weights(Wfull: np.ndarray, b: np.ndarray):
    # V[ci*8+k, dh, dw, co*8+m] = sum_j S[m,j,k] * W[j, co, ci, dh, dw]
    V = np.einsum("mjk,jcihw->ikhwcm", _S.astype(np.float64),
                  np.asarray(Wfull).astype(np.float64)).reshape(C, 3, 3, C)
    G = [V[:, 0], (V[:, 0] + V[:, 1] + V[:, 2]) / 2,
         (V[:, 0] - V[:, 1] + V[:, 2]) / 2, V[:, 2]]   # each [ic, dw, oc]
    wf = np.empty((C, WCOLS), dtype=np.float16)
    for u in range(4):
        for dw in range(3):
            wf[:, (u * 3 + dw) * C:(u * 3 + dw + 1) * C] = G[u][:, dw, :]
    bias = np.einsum("mjk,jc->cm", _S.astype(np.float64),
                     np.asarray(b).astype(np.float64)).reshape(C)
    wf[:, 12 * C] = bias.astype(np.float16)
    return np.ascontiguousarray(wf), bias.astype(np.float32).reshape(C, 1)


def _prep_inputs(x: np.ndarray) -> np.ndarray:
    # [B, C, H, W] -> Winograd row-transformed flat planes [B, C, FLAT]
    nB = x.shape[0]
    pr = np.zeros((nB, C, H + 2, W), dtype=np.float32)
    pr[:, :, 1:-1, :] = x
    xt = np.zeros((nB, C, NT, 4, PWR), dtype=np.float16)
    xt[:, :, :, 0, 1:W + 1] = pr[:, :, 0:2 * NT:2] - pr[:, :, 2:2 * NT + 2:2]
    xt[:, :, :, 1, 1:W + 1] = pr[:, :, 1:2 * NT + 1:2] - pr[:, :, 3:2 * NT + 3:2]
    xt[:, :, :, 2, 1:W + 1] = pr[:, :, 1:2 * NT + 1:2] + pr[:, :, 2:2 * NT + 2:2]
    xt[:, :, :, 3, 1:W + 1] = pr[:, :, 2:2 * NT + 2:2] - pr[:, :, 1:2 * NT + 1:2]
    return xt.reshape(nB, C, FLAT)


def kernel(x: np.ndarray, W: np.ndarray, b: np.ndarray) -> np.ndarray:
    from concourse.bass_utils import run_bass_kernel_spmd

    xt = _prep_inputs(np.ascontiguousarray(x, dtype=np.float32))
    wf, bb = _prep_weights(W, b)

    nc = _get_nc()
    in_maps = [{"xb": xt[c], "wf": wf, "bb": bb} for c in range(N_CORES)]
    res = run_bass_kernel_spmd(nc, in_maps, core_ids=list(range(N_CORES)))
    out = np.empty((N_CORES, C, H, 128), dtype=np.float32)
    for c in range(N_CORES):
        o2 = res.results[c]["out2"]          # [C, NG, 2, GPT, W]
        out[c] = o2.transpose(0, 1, 3, 2, 4).reshape(C, H, 128)
    return out
